# revision 39
# baseline (speedup 1.0000x reference)
"""Trainium2 Bass kernel for nn_AttentionBlockManual (dense transformer block).

Reference computation (per batch element n):
    temb = relu(t @ W_t.T + b_t)                      # [C]
    xin  = x + temb[:, None, None]                    # [C, H, W]
    tokens: full spatial attention over L = H*W = 1024 tokens, dim C = 256
    q/k/v = proj(xin), scores = q k^T / 16, P = softmax, o = P v
    out  = o @ Wp.T + bp, transposed back, + residual x

Token relabeling note: the reference's transpose(1,3) is a pure permutation of
the 1024 tokens applied consistently to q/k/v and inverted on output; full
softmax attention is permutation-equivariant, so we use the natural memory
order (h-major) token index and skip both transposes.

Sharding: data-parallel over batch N=32 across 8 cores (4 batches per core),
params replicated. No collectives.

fp8 formulation (all big matmuls run e4m3/e5m2 in DoubleRow perf mode, which
contracts 256 per instruction at 2x rate):
  - scores: S = xin^T (Wq^T Wk) xin, so M = Wq^T Wk is precomputed once
    (bf16 matmul of the naturally-laid-out weights -- no transposes needed)
    and a single fused projection g = M^T xin replaces both Q and K.
    S^T tile [j, i] = matmul(lhsT=x8[:, :, jt], rhs=g8[:, :, ih]).
  - exp: ACT, fp32 PSUM in -> e5m2 out (range fits e5m2; no max-subtraction
    needed), unnormalized.
  - V path is computed from x WITHOUT temb ("centered"): v and the attention
    output then have zero batch-constant component, which is what makes
    e4m3 quantization of Wv/Wp/otn accurate (a fixed relative error on the
    large temb-driven constant otherwise dominates).  The dropped constant
    contributes Wp @ (Wv @ temb) to every token, exact in bf16, and is
    folded into the output bias: out = proj(otn) + (bp + pc) + x.
  - rowsums via an all-ones e4m3 [128,2,128] DoubleRow matmul (replicated
    across partitions, so 1/rowsum needs no cross-partition broadcast);
    normalization via reciprocal_approx_fast at the O^T PSUM->SBUF mul.
  Measured end-to-end rel err vs the fp32 reference: ~7e-3.

Engine/queue balance per batch (ACT is the pacer at ~13us/batch):
  - ACT: 16 exps + the ct0 halves of the x8/x8c casts (bias-add identity).
  - DVE: g8/v8/otn casts, reciprocal, output epilogue, ct1 cast halves.
  - PE: 56 DoubleRow matmuls.  gpsimd tensor ops are ~16x slower than DVE,
    so the Pool engine only issues the store DMAs (SWDGE queue) -- keeping
    stores off the sync/scalar queues so they never block the x prefetch
    or the exps.
  - t/b_t/bp arrive via natural-layout DMA + tiny identity matmuls on the
    PE (an element-strided transpose DMA costs ~10us of descriptor time).

The score loop is software-pipelined one even/odd j-tile pair deep, carried
across the query-half boundary (the PE queue is in-order).  Tail work
(recip/otn/proj/epilogue/store) and the next batch's g/V projections are
interleaved into the score loop's emission so they execute under the
ACT-paced exp cadence instead of serializing between batches.
"""

from contextlib import ExitStack

import numpy as np

import concourse.bacc as bacc
import concourse.tile as tile
from concourse import mybir
from concourse.bass_utils import run_bass_kernel_spmd
from concourse.masks import make_identity

F32 = mybir.dt.float32
BF16 = mybir.dt.bfloat16
E4 = mybir.dt.float8e4
E5 = mybir.dt.float8e5
AF = mybir.ActivationFunctionType
ALU = mybir.AluOpType
DR = mybir.MatmulPerfMode.DoubleRow

N_CORES = 8
B = 4            # batches per core
C = 256          # channels
L = 1024         # tokens (H*W)
D = 256          # qk/v dim
T = 512          # time embedding dim
P = 128          # partitions
CT = C // P      # 2 channel chunks
DT = D // P      # 2 dim chunks
TT = T // P      # 4 time chunks
JT = L // P      # 8 key-token chunks
JP = JT // 2     # 4 key-token chunk PAIRS (DoubleRow granularity)
NH = 512         # moving-dim chunk (one PSUM bank of fp32)
IH = L // NH     # 2 query-token halves
SCALE = 1.0 / np.sqrt(256.0)


def _build_body(tc, x_d, t_d, wt_d, bt_d, wq_d, wk_d, wv_d, wp_d, bp_d, out_d):
    nc = tc.nc

    ctx = ExitStack()
    const = ctx.enter_context(tc.tile_pool(name="const", bufs=1))
    wraw = ctx.enter_context(tc.tile_pool(name="wraw", bufs=2))
    xpool = ctx.enter_context(tc.tile_pool(name="xp", bufs=3))
    x8pool = ctx.enter_context(tc.tile_pool(name="x8p", bufs=2))
    gpool = ctx.enter_context(tc.tile_pool(name="gp", bufs=2))
    vpool = ctx.enter_context(tc.tile_pool(name="vp", bufs=2))
    espool = ctx.enter_context(tc.tile_pool(name="es", bufs=4))
    otnp = ctx.enter_context(tc.tile_pool(name="otn", bufs=2))
    rbp = ctx.enter_context(tc.tile_pool(name="rb", bufs=2))
    ypool = ctx.enter_context(tc.tile_pool(name="yp", bufs=4))
    # PSUM: 5 shared 1-bank slots (scores/g/V/proj -- the deep ring lets the
    # S stream run ~2 pairs ahead of the exps), 1-bank rowsum, 2-bank O^T.
    # rs/ot are single-buffered: the ih1 accumulation group's first write is
    # emitted after ih0's recip/otn reads, so reuse is safe by emission order.
    pss = ctx.enter_context(tc.tile_pool(name="pss", bufs=5, space="PSUM"))
    rsp = ctx.enter_context(tc.tile_pool(name="rsp", bufs=1, space="PSUM"))
    psot = ctx.enter_context(tc.tile_pool(name="psot", bufs=1, space="PSUM"))

    # ---- constants FIRST: the identity masks are built by gpsimd, and
    # emitting them before the gpsimd DMA issues keeps every PE transpose
    # off the DMA critical path.
    ident = const.tile([P, P], F32, tag="ident")
    make_identity(nc, ident)
    id4 = const.tile([B, B], F32, tag="id4")
    make_identity(nc, id4)
    id2 = const.tile([CT, CT], F32, tag="id2")
    make_identity(nc, id2)
    ones8 = const.tile([P, 2, P], E4, tag="ones8")
    nc.vector.memset(ones8, 1.0)

    # ---- input DMAs ----------------------------------------------------
    # ~100 GB/s effective per queue, so order each queue by when the data
    # gates compute: temb chain needs W_t chunk 0 + t + b_t first; x0 and
    # Wq/Wk gate batch 0's x8/g; Wv by the first PV; Wp/bp only by +10us.
    t_nat = wraw.tile([B, T], F32, tag="tnat")
    wt_raw = wraw.tile([P, CT, T], F32, tag="wtr")
    bt_nat = wraw.tile([CT, P], F32, tag="btn")
    bp_nat = wraw.tile([CT, P], F32, tag="bpn")
    wv_raw = wraw.tile([P, DT, C], F32, tag="wvr")
    wp_raw = wraw.tile([P, CT, D], F32, tag="wpr")
    wq_nat = wraw.tile([P, DT, C], F32, tag="wqn")
    wk_nat = wraw.tile([P, DT, C], F32, tag="wkn")
    x0_sb = xpool.tile([P, CT, L], F32, tag="x")

    nc.sync.dma_start(out=t_nat, in_=t_d)
    nc.sync.dma_start(out=wt_raw[:, 0, :], in_=wt_d[0:P, :])
    nc.sync.dma_start(out=x0_sb[:, 0, :],
                      in_=x_d[0, 0:P, :, :].rearrange("c h w -> c (h w)"))

    nc.scalar.dma_start(out=wt_raw[:, 1, :], in_=wt_d[P:2 * P, :])
    nc.scalar.dma_start(out=x0_sb[:, 1, :],
                        in_=x_d[0, P:2 * P, :, :].rearrange("c h w -> c (h w)"))

    nc.gpsimd.dma_start(out=bt_nat, in_=bt_d.rearrange("(a p) -> a p", p=P))
    nc.gpsimd.dma_start(out=bp_nat, in_=bp_d.rearrange("(a p) -> a p", p=P))
    for a in range(DT):
        nc.gpsimd.dma_start(out=wq_nat[:, a, :], in_=wq_d[a * P:(a + 1) * P, :])
        nc.gpsimd.dma_start(out=wk_nat[:, a, :], in_=wk_d[a * P:(a + 1) * P, :])
    for a in range(DT):
        nc.gpsimd.dma_start(out=wv_raw[:, a, :], in_=wv_d[a * P:(a + 1) * P, :])
    for a in range(CT):
        nc.gpsimd.dma_start(out=wp_raw[:, a, :], in_=wp_d[a * P:(a + 1) * P, :])

    # ---- small transposes via identity matmuls -------------------------
    # t [B, T] -> t_all_bf [128, TT, B];  b_t/bp [CT, 128] -> [128, CT]
    t_all_bf = const.tile([P, TT, B], BF16, tag="tallbf")
    for kt in range(TT):
        ps = pss.tile([P, B], F32, tag="ps")
        nc.tensor.matmul(ps, t_nat[:, kt * P:(kt + 1) * P], id4,
                         start=True, stop=True)
        nc.vector.tensor_copy(out=t_all_bf[:, kt, :], in_=ps)
    bt_sb = const.tile([P, CT], F32, tag="bt")
    bp_sb = const.tile([P, CT], F32, tag="bp")
    for src, dst in ((bt_nat, bt_sb), (bp_nat, bp_sb)):
        ps = pss.tile([P, CT], F32, tag="ps")
        nc.tensor.matmul(ps, src, id2, start=True, stop=True)
        nc.vector.tensor_copy(out=dst, in_=ps)

    # ---- temb chain first (it gates batch 0's x8) ----------------------
    wtT = const.tile([P, TT, C], BF16, tag="wtT")
    for a in range(CT):
        for b in range(TT):
            ps = pss.tile([P, P], F32, tag="ps")
            nc.tensor.transpose(ps, wt_raw[:, a, b * P:(b + 1) * P], ident)
            if (a * TT + b) % 2 == 0:
                nc.scalar.copy(out=wtT[:, b, a * P:(a + 1) * P], in_=ps)
            else:
                nc.vector.tensor_copy(out=wtT[:, b, a * P:(a + 1) * P], in_=ps)
    # temb for all batches + bf16 copy (for the pc matmuls)
    temb_all = const.tile([P, CT, B], F32, tag="temba")
    temb_bf = const.tile([P, CT, B], BF16, tag="tembbf")
    for ct in range(CT):
        tb_ps = pss.tile([P, B], F32, tag="ps")
        for kt in range(TT):
            nc.tensor.matmul(tb_ps, wtT[:, kt, ct * P:(ct + 1) * P],
                             t_all_bf[:, kt, :], start=(kt == 0), stop=(kt == TT - 1))
        nc.scalar.activation(out=temb_all[:, ct, :], in_=tb_ps, func=AF.Relu,
                             bias=bt_sb[:, ct:ct + 1], scale=1.0)
        nc.vector.tensor_copy(out=temb_bf[:, ct, :], in_=temb_all[:, ct, :])

    # ---- M = Wq^T Wk  (bf16 matmul; M rows on partitions) --------------
    wq_bf = const.tile([P, DT, C], BF16, tag="wqbf")
    wk_bf = const.tile([P, DT, C], BF16, tag="wkbf")
    nc.vector.tensor_copy(out=wq_bf, in_=wq_nat)
    nc.vector.tensor_copy(out=wk_bf, in_=wk_nat)
    m8 = const.tile([P, CT, C], E4, tag="m8")
    for cm in range(CT):
        ps = pss.tile([P, C], F32, tag="ps")
        for kd in range(DT):
            nc.tensor.matmul(ps, wq_bf[:, kd, cm * P:(cm + 1) * P],
                             wk_bf[:, kd, :], start=(kd == 0), stop=(kd == DT - 1))
        nc.vector.tensor_copy(out=m8[:, cm, :], in_=ps)

    # ---- deferred weight prep: emitted into batch 0's score loop -------
    wvT_bf = const.tile([P, CT, D], BF16, tag="wvTbf")
    wv8T = const.tile([P, CT, D], E4, tag="wv8T")
    wpT_bf = const.tile([P, DT, C], BF16, tag="wpTbf")
    wp8T = const.tile([P, DT, C], E4, tag="wp8T")
    vtmp_bf = const.tile([P, DT, B], BF16, tag="vtmp")
    pcb_all = const.tile([P, CT, B], F32, tag="pcb")

    def emit_wv_prep():
        for a in range(DT):
            for b in range(CT):
                ps = pss.tile([P, P], F32, tag="ps")
                nc.tensor.transpose(ps, wv_raw[:, a, b * P:(b + 1) * P], ident)
                nc.scalar.copy(out=wvT_bf[:, b, a * P:(a + 1) * P], in_=ps)
                nc.vector.tensor_copy(out=wv8T[:, b, a * P:(a + 1) * P], in_=ps)

    def emit_wp_prep():
        for a in range(CT):
            for b in range(DT):
                ps = pss.tile([P, P], F32, tag="ps")
                nc.tensor.transpose(ps, wp_raw[:, a, b * P:(b + 1) * P], ident)
                nc.scalar.copy(out=wpT_bf[:, b, a * P:(a + 1) * P], in_=ps)
                nc.vector.tensor_copy(out=wp8T[:, b, a * P:(a + 1) * P], in_=ps)

    def emit_vtmp():
        for m in range(DT):
            ps = pss.tile([P, B], F32, tag="ps")
            for kc in range(CT):
                nc.tensor.matmul(ps, wvT_bf[:, kc, m * P:(m + 1) * P],
                                 temb_bf[:, kc, :], start=(kc == 0),
                                 stop=(kc == CT - 1))
            nc.vector.tensor_copy(out=vtmp_bf[:, m, :], in_=ps)

    def emit_pcb():
        """pcb = bp + Wp @ (Wv @ temb): the centered-V add-back bias."""
        for ct in range(CT):
            ps = pss.tile([P, B], F32, tag="ps")
            for kd in range(DT):
                nc.tensor.matmul(ps, wpT_bf[:, kd, ct * P:(ct + 1) * P],
                                 vtmp_bf[:, kd, :], start=(kd == 0),
                                 stop=(kd == DT - 1))
            nc.vector.tensor_scalar_add(pcb_all[:, ct, :], ps, bp_sb[:, ct:ct + 1])

    # ---- per-batch pipeline pieces --------------------------------------
    state = {0: dict(x_sb=x0_sb)}

    def load_x(n):
        if n >= B:
            return
        x_sb = xpool.tile([P, CT, L], F32, tag="x")
        for ct in range(CT):
            nc.sync.dma_start(
                out=x_sb[:, ct, :],
                in_=x_d[n, ct * P:(ct + 1) * P, :, :].rearrange("c h w -> c (h w)"))
        state[n] = dict(x_sb=x_sb)

    def emit_x8(n):
        """x8c = e4(x), then x8 = x8c + temb_n (all-8/16-bit DVE op).
        x8c's ct0 half runs on ACT, the rest on DVE.  For batch 0 the x8
        halves are built directly from x instead (one less serial hop on
        the startup critical path)."""
        if n >= B:
            return
        s = state[n]
        x_sb = s["x_sb"]
        x8 = x8pool.tile([P, CT, L], E4, tag="x8")
        x8c = x8pool.tile([P, CT, L], E4, tag="x8c")
        nc.scalar.copy(out=x8c[:, 0, :], in_=x_sb[:, 0, :])
        nc.vector.tensor_copy(out=x8c[:, 1, :], in_=x_sb[:, 1, :])
        if n == 0:
            nc.scalar.activation(out=x8[:, 0, :], in_=x_sb[:, 0, :],
                                 func=AF.Identity, bias=temb_all[:, 0, 0:1])
            nc.vector.tensor_scalar_add(x8[:, 1, :], x_sb[:, 1, :],
                                        temb_all[:, 1, 0:1])
        else:
            for ct in range(CT):
                nc.vector.tensor_scalar_add(x8[:, ct, :], x8c[:, ct, :],
                                            temb_all[:, ct, n:n + 1])
        s["x8"], s["x8c"] = x8, x8c

    def emit_g(n, cm):
        """Fused q/k projection g = M^T xin (one of CT column chunks)."""
        if n >= B:
            return
        s = state[n]
        if cm == 0:
            g8_new = gpool.tile([P, CT, L], E4, tag="g8")
            s["g8"] = g8_new
        g8 = s["g8"]
        for nh in range(IH):
            ps = pss.tile([P, NH], F32, tag="ps")
            nc.tensor.matmul(ps, m8[:, :, cm * P:(cm + 1) * P],
                             s["x8"][:, :, nh * NH:(nh + 1) * NH],
                             start=True, stop=True, perf_mode=DR)
            nc.vector.tensor_copy(out=g8[:, cm, nh * NH:(nh + 1) * NH], in_=ps)

    def emit_v(n, vhalf):
        """Centered V^T (two of four jt-pairs); a pair shares one PSUM bank."""
        if n >= B:
            return
        s = state[n]
        if vhalf == 0:
            vt8_new = vpool.tile([P, JT, D], E4, tag="vt8")
            s["vt8"] = vt8_new
        vt8 = s["vt8"]
        for jp in (2 * vhalf, 2 * vhalf + 1):
            ps = pss.tile([P, NH], F32, tag="ps")
            for half in range(2):
                jt = 2 * jp + half
                nc.tensor.matmul(ps[:, half * D:(half + 1) * D],
                                 s["x8c"][:, :, jt * P:(jt + 1) * P], wv8T,
                                 start=True, stop=True, perf_mode=DR)
            nc.vector.tensor_copy(out=vt8[:, 2 * jp:2 * jp + 2, :], in_=ps)

    def emit_recip_otn(n, ih):
        s = state[n]
        recip_b = rbp.tile([P, NH], F32, tag="recipb")
        nc.vector.reciprocal_approx_fast(out=recip_b, in_=s["rs_list"][ih])
        otn = otnp.tile([P, DT, NH], E4, tag="otn")
        for dh in range(DT):
            nc.vector.tensor_mul(otn[:, dh, :], s["ot_list"][ih][:, dh, :], recip_b)
        s.setdefault("otn", {})[ih] = otn

    def emit_tails_pe(n, ih):
        """Projection + epilogue + store for one query half."""
        s = state[n]
        isl = slice(ih * NH, (ih + 1) * NH)
        otn = s["otn"][ih]
        for ct in range(CT):
            pj_ps = pss.tile([P, NH], F32, tag="ps")
            nc.tensor.matmul(pj_ps, wp8T[:, :, ct * P:(ct + 1) * P], otn,
                             start=True, stop=True, perf_mode=DR)
            y = ypool.tile([P, NH], F32, tag="y")
            nc.vector.scalar_tensor_tensor(
                out=y, in0=pj_ps, scalar=pcb_all[:, ct, n:n + 1],
                in1=s["x_sb"][:, ct, isl], op0=ALU.add, op1=ALU.add,
            )
            # stores go out on the pool SWDGE queue (keeps sync/scalar free
            # for loads/exps) -- except the LAST batch, whose stores would
            # otherwise drain serially after the compute: sync/scalar are
            # idle by then and all exps are already queued ahead.  Those
            # final stores are split into quarter-tiles across both queues
            # so the last transfer is as short as possible.
            dst = (out_d[n, ct * P:(ct + 1) * P, :, :]
                   .rearrange("c h w -> c (h w)")[:, isl])
            if n == B - 1:
                hh = NH // 2
                for q in range(2):
                    (nc.sync if q == 0 else nc.scalar).dma_start(
                        out=dst[:, q * hh:(q + 1) * hh],
                        in_=y[:, q * hh:(q + 1) * hh],
                    )
            else:
                nc.gpsimd.dma_start(out=dst, in_=y)

    def emit_rs_pv(n, ih, jp, es):
        s = state[n]
        vt8 = s["vt8"]
        nc.tensor.matmul(s["rs_list"][ih], ones8, es,
                         start=(jp == 0), stop=(jp == JP - 1), perf_mode=DR)
        for dh in range(DT):
            nc.tensor.matmul(
                s["ot_list"][ih][:, dh, :],
                vt8[:, 2 * jp:2 * jp + 2, dh * P:(dh + 1) * P],
                es, start=(jp == 0), stop=(jp == JP - 1), perf_mode=DR,
            )

    def emit_scores(n):
        s = state[n]
        load_x(n + 1)
        x8, g8 = s["x8"], s["g8"]
        rs_list, ot_list = [], []
        for _ih in range(IH):
            rs_ps = rsp.tile([P, NH], F32, tag="rs")
            ot_ps = psot.tile([P, DT, NH], F32, tag="ot")
            rs_list.append(rs_ps)
            ot_list.append(ot_ps)
        s["rs_list"], s["ot_list"] = rs_list, ot_list

        pend = []
        for ih in range(IH):
            isl = slice(ih * NH, (ih + 1) * NH)
            for jp in range(JP):
                es = espool.tile([P, 2, NH], E5, tag="es")
                for half in range(2):
                    jt = 2 * jp + half
                    st_ps = pss.tile([P, NH], F32, tag="ps")
                    nc.tensor.matmul(st_ps, x8[:, :, jt * P:(jt + 1) * P],
                                     g8[:, :, isl], start=True, stop=True,
                                     perf_mode=DR)
                    nc.scalar.activation(out=es[:, half, :], in_=st_ps,
                                         func=AF.Exp, scale=SCALE)
                pend.append((n, ih, jp, es))
                if len(pend) > 2:
                    emit_rs_pv(*pend.pop(0))
                # Interleave tail / next-batch work under the exp cadence,
                # a few matmuls per slot so the PE never starves the ACT exp
                # stream for long.  NOTE emission order IS dependency order:
                # ih0's rowsum accumulation only closes at the rs/pv flush
                # during ih1-p1 (2-pair-deep pend), so recip/otn(ih0) come
                # after that point.  PSUM-reading DVE work (recip/otn, the
                # epilogue) is deliberately bunched near the batch boundary:
                # overlapping it with the exp stream slows every ACT PSUM
                # read by ~20% (port contention), which costs more than the
                # boundary bubble it would hide.
                if ih == 0 and jp == 0 and n == 0:
                    emit_v(0, 0)      # cold start: batch 0's V inline
                    emit_v(0, 1)
                if ih == 0 and jp == 1:
                    # x(n+1)'s DMA (issued at the top of this batch) has
                    # landed by now, so these don't stall the in-order
                    # ACT/DVE queues.
                    emit_x8(n + 1)
                    if n == 0:
                        emit_wp_prep()
                if ih == 0 and jp == 2:
                    if n > 0:
                        emit_tails_pe(n - 1, 1)
                    else:
                        emit_vtmp()
                if ih == 0 and jp == 3:
                    emit_g(n + 1, 0)
                    if n == 0:
                        emit_pcb()
                if ih == 1 and jp == 0:
                    emit_g(n + 1, 1)
                if ih == 1 and jp == 1:
                    emit_recip_otn(n, 0)
                    emit_v(n + 1, 0)
                if ih == 1 and jp == 2:
                    emit_v(n + 1, 1)
                if ih == 1 and jp == 3:
                    emit_tails_pe(n, 0)
        while pend:
            emit_rs_pv(*pend.pop(0))
        emit_recip_otn(n, 1)

    emit_x8(0)
    emit_g(0, 0)
    emit_g(0, 1)
    emit_wv_prep()
    for n in range(B):
        emit_scores(n)
    emit_tails_pe(B - 1, 1)

    ctx.close()


_CACHE = {}


def _get_program():
    if "nc" in _CACHE:
        return _CACHE["nc"]
    nc = bacc.Bacc("TRN2", target_bir_lowering=False, debug=False,
                   num_devices=N_CORES)
    x_d = nc.dram_tensor("x", [B, C, 32, 32], F32, kind="ExternalInput").ap()
    t_d = nc.dram_tensor("t", [B, T], F32, kind="ExternalInput").ap()
    wt_d = nc.dram_tensor("W_t", [C, T], F32, kind="ExternalInput").ap()
    bt_d = nc.dram_tensor("b_t", [C], F32, kind="ExternalInput").ap()
    wq_d = nc.dram_tensor("Wq", [D, C], F32, kind="ExternalInput").ap()
    wk_d = nc.dram_tensor("Wk", [D, C], F32, kind="ExternalInput").ap()
    wv_d = nc.dram_tensor("Wv", [D, C], F32, kind="ExternalInput").ap()
    wp_d = nc.dram_tensor("Wp", [C, D], F32, kind="ExternalInput").ap()
    bp_d = nc.dram_tensor("bp", [C], F32, kind="ExternalInput").ap()
    out_d = nc.dram_tensor("out", [B, C, 32, 32], F32, kind="ExternalOutput").ap()

    with tile.TileContext(nc) as tc:
        _build_body(tc, x_d, t_d, wt_d, bt_d, wq_d, wk_d, wv_d, wp_d, bp_d, out_d)
    nc.compile()
    _CACHE["nc"] = nc
    return nc


def _run(inputs, trace=False, tmpdir=None):
    nc = _get_program()
    x = np.ascontiguousarray(np.asarray(inputs["x"], dtype=np.float32))
    t = np.ascontiguousarray(np.asarray(inputs["t"], dtype=np.float32))
    rep = {
        k: np.ascontiguousarray(np.asarray(inputs[k], dtype=np.float32))
        for k in ("W_t", "b_t", "Wq", "Wk", "Wv", "Wp", "bp")
    }
    in_maps = []
    for i in range(N_CORES):
        m = {"x": x[i * B:(i + 1) * B], "t": t[i * B:(i + 1) * B]}
        m.update(rep)
        in_maps.append(m)
    res = run_bass_kernel_spmd(nc, in_maps, list(range(N_CORES)),
                               trace=trace, tmpdir=tmpdir)
    out = np.concatenate([res.results[i]["out"] for i in range(N_CORES)], axis=0)
    return out, res


def kernel(**inputs):
    out, _ = _run(inputs)
    return out
